# revision 3
# baseline (speedup 1.0000x reference)
"""Trainium2 Bass kernel for nn_Att_76381698392129 (fp8 DoubleRow).

kernel(**inputs) -> np.ndarray, self-contained.

Reference math:
    v     = x @ value_w.T                      [B, N, 3]
    score = (key_w @ query_w) / 16             [N, N]
    l1    = sum_o |score[i, o]|
    s_n   = score / max(l1, 1e-12)
    y     = einsum("io,bid->bod", s_n, v)      [B, N, 3]

Factored algorithm (never materializes the N x N score matrix):
    raw_l1[i] = sum_o |(key_w @ query_w)[i, o]|         (the only big matmul)
    r[i]      = 1 / max(raw_l1[i], 1.6e-11)             (the /16 scale cancels)
    T         = key_w.T @ (X * r)       [H, B*3],  X[i, (b,d)] = x[b, i, d]
    Tv[h,(b,e)] = sum_d T[h,(b,d)] vw[e,d]              (3x3 value map)
    y[b,o,e]  = (query_w.T @ Tv)[o, (b,e)]

Distribution (8 NeuronCores), row (i) sharding:
  Phase A - each core computes raw_l1 for its 640 rows and the partial
  T_c = key_w[shard].T @ (X[shard] * r)  [256, 192].  The score matmul
  runs in fp8-e4m3 with DoubleRow perf mode (K=256 per instruction);
  inputs are pre-scaled by 64 on the host and the 64^2 factor is folded
  into the X operand, so no on-device compensation is needed.  The
  |score| row-sum is split between the Activation engine (Abs+accum)
  and the Vector engine (tensor_reduce abs) reading PSUM.
  Host glue: sums the 8 partial T_c (the gather step of the contraction
  sharding) and applies the 3x3 value map to the tiny [256,192] sum.
  Phase B - output rows (o) sharded 8 ways: Y[o-shard] = qw[:, shard].T
  @ Tv in bf16.  Each phase runs as 8 single-device executions.
"""

import os
from contextlib import ExitStack

import numpy as np

import concourse.bass as bass
import concourse.mybir as mybir
import concourse.tile as tile

F32 = mybir.dt.float32
BF16 = mybir.dt.bfloat16
FP8 = mybir.dt.float8e4
AX = mybir.AxisListType
ALU = mybir.AluOpType
ACTF = mybir.ActivationFunctionType
DR = mybir.MatmulPerfMode.DoubleRow

N = 5023
H_DIM = 256
B = 64
BD = B * 3
N_CORES = 8
N_PAD = 5120
S = N_PAD // N_CORES     # 640 rows per core
MT = S // 128            # 5 row tiles
OC = 512                 # o columns per matmul
NCH = N_PAD // OC        # 10 o-chunks
TW = 1024                # PSUM tile width (2 chunks)
NT = NCH * OC // TW      # 5 psum tiles per row tile
LASTW = N - (NCH - 1) * OC  # valid cols in the last chunk (415; rest are pad)
SCALE = 64.0             # fp8 pre-scale (host); 64^2 folded into xs
EPS_DEV = 1.6e-11 * SCALE * SCALE  # clamp in device units

LAST_HW_EXEC_NS = None
LAST_PHASE_A_NS = None
LAST_PHASE_B_NS = None

_PATCHED = False


def _patch_tile_drain():
    """This walrus build rejects >1 sync-wait on an InstDrain; re-emit the
    final drain's waits as individual wait_ge instructions."""
    global _PATCHED
    if _PATCHED:
        return
    _PATCHED = True
    import bass_rust

    def _drain_and_barrier(self, tick_clock, wait_clock):
        nc = self.nc
        probe = nc.sync.nop(nofuse=True, hint="drain_waits")
        wait_clock.add_sem_waits(
            probe.ins, bass_rust.ScopedClock({None: tick_clock.global_clock})
        )
        waits = list(probe.ins.sync_info.on_wait or []) if probe.ins.sync_info else []
        if probe.ins.sync_info is not None:
            probe.ins.sync_info.on_wait = []
        handles = {h.num: h for h in self.sems.allocated().values()}
        for w in waits:
            h = handles.get(w.id)
            assert h is not None, f"no handle for sem wait {w}"
            assert w.wait_mode == "sem-ge-imm", w
            nc.sync.wait_ge(h, w.wait_value)
        nc.sync.drain()
        nc.all_engine_barrier()
        popped = nc._tile_sem_poison_stack.pop()
        assert popped is self._sem_poison
        nc.clear_and_free_semaphores(list(self.sems.allocated().values()))
        nc.all_engine_barrier()

    tile.TileContext._drain_and_barrier = _drain_and_barrier


def _fix_multiwait(nc, max_waits=1):
    """This walrus build accepts at most one sync-wait command per
    instruction; peel extra waits onto same-engine nops just ahead."""
    f = nc.m.functions[0]
    all_blocks = list(f.blocks)
    for blk in all_blocks:
        insts = blk.instructions
        new = []
        for inst in insts:
            si = inst.sync_info
            w = list(si.on_wait) if si and si.on_wait else []
            if len(w) > max_waits:
                keep = w[-max_waits:]
                for extra in w[:-max_waits]:
                    nop = nc.engines[inst.engine].nop(
                        nofuse=True, hint="waitfix").ins
                    removed = False
                    for b2 in all_blocks:
                        l2 = b2.instructions
                        for k in range(len(l2) - 1, -1, -1):
                            if l2[k] is nop:
                                del l2[k]
                                removed = True
                                break
                        if removed:
                            break
                    assert removed, "waitfix nop not found in any block"
                    if nop.sync_info is None:
                        nop.sync_info = mybir.SyncInfo(on_wait=[extra],
                                                       on_update=[])
                    else:
                        nop.sync_info.on_wait = [extra]
                    new.append(nop)
                si.on_wait = keep
            new.append(inst)
        insts[:] = new
    return nc


def _build_phase_a():
    nc = bass.Bass("TRN2", target_bir_lowering=False, debug=False)
    # blob0 = kwt8 [128, 2*S] ++ first 2 qw chunk-pairs [128, 2*2*OC], fp8,
    # one DMA so the first matmuls are gated by a single transfer.
    blob0_d = nc.dram_tensor("blob0", [128, 2 * S + 4 * OC], FP8,
                             kind="ExternalInput")
    # chunk-major qw8 tail: col = (c-2)*1024 + j*512 + o
    qw8_d = nc.dram_tensor("qw8", [128, (NCH - 2) * 2 * OC], FP8,
                           kind="ExternalInput")
    xs_d = nc.dram_tensor("xs", [128, MT * BD], F32, kind="ExternalInput")
    kwb_d = nc.dram_tensor("kwb", [128, MT * H_DIM], BF16, kind="ExternalInput")
    tv_d = nc.dram_tensor("tv", [128, 2 * BD], BF16, kind="ExternalOutput")

    with tile.TileContext(nc) as tc, ExitStack() as ctx:
        sb = ctx.enter_context(tc.tile_pool(name="sb", bufs=1))
        scr_pool = ctx.enter_context(tc.tile_pool(name="scr", bufs=2))
        ps_pool = ctx.enter_context(tc.tile_pool(name="ps", bufs=3, space="PSUM"))
        t_pool = ctx.enter_context(tc.tile_pool(name="tps", bufs=1, space="PSUM"))

        blob0 = sb.tile([128, 2 * S + 4 * OC], FP8, name="blob0", tag="blob0")
        # gating transfers concurrently: kwt8 on SP, chunk 0 via SWDGE,
        # chunk 1 on SP behind kwt8
        nc.sync.dma_start(blob0[:, 0:2 * S], blob0_d.ap()[:, 0:2 * S])
        nc.gpsimd.dma_start(blob0[:, 2 * S:2 * S + 2 * OC],
                            blob0_d.ap()[:, 2 * S:2 * S + 2 * OC])
        nc.sync.dma_start(blob0[:, 2 * S + 2 * OC:],
                          blob0_d.ap()[:, 2 * S + 2 * OC:])
        kwt8 = blob0[:, 0:2 * S]

        qw8 = sb.tile([128, (NCH - 2) * 2 * OC], FP8, name="qw8", tag="qw8")
        # remaining 8 chunks as 4 chunk-pair DMAs, alternating generation
        # queues (SP/HWDGE vs Pool/SWDGE) so pairs land back-to-back
        for g in range(4):
            eng = nc.sync if g % 2 == 0 else nc.gpsimd
            eng.dma_start(qw8[:, g * 4 * OC:(g + 1) * 4 * OC],
                          qw8_d.ap()[:, g * 4 * OC:(g + 1) * 4 * OC])

        # xs/kwb are only needed from the first normalize (~12us in); issue
        # after the qw8 stream so their transfers don't delay it
        xs_all = sb.tile([128, MT * BD], F32, name="xs_all", tag="xs_all")
        nc.gpsimd.dma_start(xs_all[:], xs_d.ap())
        kwb = sb.tile([128, MT * H_DIM], BF16, name="kwb", tag="kwb")
        nc.gpsimd.dma_start(kwb[:], kwb_d.ap())

        kwt8_r = kwt8.rearrange("p (j i) -> p j i", j=2)

        def qw_chunk(c):
            if c < 2:
                base = 2 * S + c * 2 * OC
                ap = blob0[:, base:base + 2 * OC]
            else:
                base = (c - 2) * 2 * OC
                ap = qw8[:, base:base + 2 * OC]
            return ap.rearrange("p (j o) -> p j o", j=2)

        # two T accumulation groups must live in separate PSUM zero
        # regions (2KB banks); place h-blocks 512 floats apart
        t_ps = t_pool.tile([128, 1024], F32, name="tps", tag="tps")

        # o-outer / m-inner: each arriving qw chunk-pair feeds all 5 row
        # tiles, so the DMA stream fully overlaps the reduce stream.
        parts = [scr_pool.tile([128, NT + 1], F32, name=f"part{m}",
                               tag=f"part{m}") for m in range(MT)]
        for t in range(NT):
            for m in range(MT):
                # matmuls always full width (PE has slack; pad cols are 0);
                # the reduces skip the 97 zero pad columns of the last chunk
                tw = TW if t < NT - 1 else OC + LASTW
                ps = ps_pool.tile([128, TW], F32, name="ps", tag="ps")
                for k in range(TW // OC):
                    nc.tensor.matmul(
                        ps[:, k * OC:(k + 1) * OC],
                        kwt8_r[:, :, m * 128:(m + 1) * 128],
                        qw_chunk(t * (TW // OC) + k),
                        start=True, stop=True, perf_mode=DR,
                    )
                if (t + m) % 2 == 0:
                    scr = scr_pool.tile([128, TW], BF16, name="scr", tag="scr")
                    nc.scalar.activation(
                        scr[:, 0:tw], ps[:, 0:tw], ACTF.Abs,
                        accum_out=parts[m][:, t:t + 1])
                else:
                    nc.vector.tensor_reduce(
                        parts[m][:, t:t + 1], ps[:, 0:tw], axis=AX.X, op=ALU.add,
                        apply_absolute_value=True)

                if t == NT - 1:
                    # this row tile is fully reduced: normalize + T matmul.
                    # The very last m runs its chain inline on DVE (shorter
                    # critical path); earlier ms go via the idle Pool engine.
                    l1 = scr_pool.tile([128, 1], F32, name="l1", tag="l1")
                    nc.vector.tensor_reduce(
                        l1[:], parts[m][:, 0:NT], axis=AX.X, op=ALU.add)
                    nc.vector.tensor_scalar_max(l1[:], l1[:], EPS_DEV)
                    r = scr_pool.tile([128, 1], F32, name="r", tag="r")
                    nc.vector.reciprocal(r[:], l1[:])
                    xsc = scr_pool.tile([128, BD], BF16, name="xsc", tag="xsc")
                    if m == MT - 1:
                        nc.vector.tensor_scalar_mul(
                            xsc[:], xs_all[:, m * BD:(m + 1) * BD], r[:])
                    else:
                        nc.gpsimd.tensor_scalar_mul(
                            xsc[:], xs_all[:, m * BD:(m + 1) * BD], r[:])
                    for h in range(2):
                        nc.tensor.matmul(
                            t_ps[:, h * 512:h * 512 + BD],
                            kwb[:, m * H_DIM + h * 128:m * H_DIM + (h + 1) * 128],
                            xsc[:],
                            start=(m == 0),
                            stop=(m == MT - 1),
                        )

        tv_sb = sb.tile([128, 2 * BD], BF16, name="tv_sb", tag="tv_sb")
        nc.vector.tensor_copy(
            tv_sb[:].rearrange("p (h d) -> p h d", h=2),
            t_ps[:].rearrange("p (h d) -> p h d", h=2)[:, :, 0:BD])
        nc.sync.dma_start(tv_d.ap(), tv_sb[:])

    return _fix_multiwait(nc)


def _build_phase_b():
    nc = bass.Bass("TRN2", target_bir_lowering=False, debug=False)
    # blob = ts [128, 2*BD] ++ qwyb-ot0 [128, 256]  (one gating DMA)
    # qwyb tail ot-major: col = (ot-1)*256 + j*128 + o_local
    blob_d = nc.dram_tensor("blobb", [128, 2 * BD + 256], BF16,
                            kind="ExternalInput")
    qwyb_d = nc.dram_tensor("qwyb", [128, 2 * S - 256], BF16,
                            kind="ExternalInput")
    y_d = nc.dram_tensor("y", [128, MT * BD], BF16, kind="ExternalOutput")

    with tile.TileContext(nc) as tc, ExitStack() as ctx:
        sb = ctx.enter_context(tc.tile_pool(name="sb", bufs=1))
        ps_pool = ctx.enter_context(tc.tile_pool(name="ps", bufs=4, space="PSUM"))

        blob = sb.tile([128, 2 * BD + 256], BF16, name="blobb", tag="blobb")
        nc.sync.dma_start(blob[:], blob_d.ap())
        ts = blob[:, 0:2 * BD]
        qwyb = sb.tile([128, 2 * S - 256], BF16, name="qwyb", tag="qwyb")
        nc.gpsimd.dma_start(qwyb[:], qwyb_d.ap())

        def qwy_slice(ot, j):
            if ot == 0:
                return blob[:, 2 * BD + j * 128:2 * BD + (j + 1) * 128]
            base = (ot - 1) * 256 + j * 128
            return qwyb[:, base:base + 128]

        ysb = sb.tile([128, MT * BD], BF16, name="ysb", tag="ysb")
        for ot in range(MT):
            yp = ps_pool.tile([128, BD], F32, name="yp", tag="yp")
            for j in range(2):
                nc.tensor.matmul(
                    yp[:],
                    qwy_slice(ot, j),
                    ts[:, j * BD:(j + 1) * BD],
                    start=(j == 0),
                    stop=(j == 1),
                )
            if ot % 2 == 0:
                nc.vector.tensor_copy(ysb[:, ot * BD:(ot + 1) * BD], yp[:])
            else:
                nc.scalar.activation(
                    ysb[:, ot * BD:(ot + 1) * BD], yp[:], ACTF.Copy)
        nc.sync.dma_start(y_d.ap(), ysb[:])

    return _fix_multiwait(nc)


_NC_A = None
_NC_B = None


def _get_programs():
    global _NC_A, _NC_B
    if _NC_A is None:
        _patch_tile_drain()
        _NC_A = _build_phase_a()
        _NC_B = _build_phase_b()
    return _NC_A, _NC_B


def _run_phase(nc, in_maps, profile):
    """Run one SPMD phase as 8 independent single-device executions."""
    import time

    import jax
    from concourse import bass2jax

    devices = jax.devices()[:len(in_maps)]
    results = []
    max_ns = None
    if profile:
        try:
            from concourse.bass_utils import run_bass_kernel_spmd
            for d, (dev, in_map) in enumerate(zip(devices, in_maps)):
                with jax.default_device(dev):
                    r = run_bass_kernel_spmd(
                        nc, [in_map], core_ids=[0], trace=True, trace_cores=[d])
                    results.append(r.results[0])
                    if r.exec_time_ns is not None:
                        max_ns = max(max_ns or 0, r.exec_time_ns)
            return results, max_ns
        except (ImportError, ModuleNotFoundError):
            results = []
    for dev, in_map in zip(devices, in_maps):
        with jax.default_device(dev):
            try:
                results.append(
                    bass2jax.run_bass_via_pjrt(nc, [in_map], n_cores=1)[0])
            except Exception:
                # transient device hiccup: retry once after a short pause
                time.sleep(2.0)
                results.append(
                    bass2jax.run_bass_via_pjrt(nc, [in_map], n_cores=1)[0])
    return results, max_ns


def kernel(x, key_w, query_w, value_w):
    global LAST_HW_EXEC_NS, LAST_PHASE_A_NS, LAST_PHASE_B_NS
    import ml_dtypes
    FP8NP = ml_dtypes.float8_e4m3
    BF16NP = ml_dtypes.bfloat16

    x = np.asarray(x, dtype=np.float32)
    key_w = np.asarray(key_w, dtype=np.float32)
    query_w = np.asarray(query_w, dtype=np.float32)
    value_w = np.asarray(value_w, dtype=np.float32)

    profile = os.environ.get("ATT_PROFILE", "0") == "1"
    nc_a, nc_b = _get_programs()

    # ---- host-side sharding / layout prep ----
    kw_pad = np.zeros((N_PAD, H_DIM), np.float32)
    kw_pad[:N] = key_w
    qw_pad = np.zeros((H_DIM, N_PAD), np.float32)
    qw_pad[:, :N] = query_w

    # qw8 full, chunk-major [128, c*1024 + j*512 + o]
    qw_s = (qw_pad * SCALE).astype(FP8NP)            # [256, 5120]
    qw8_full = np.ascontiguousarray(
        qw_s.reshape(2, 128, NCH, OC).transpose(1, 2, 0, 3).reshape(128, -1))

    # x transposed to [N_pad, B*3], pre-scaled by 64^2
    x_pad = np.zeros((N_PAD, BD), np.float32)
    x_pad[:N] = np.ascontiguousarray(x.transpose(1, 0, 2)).reshape(N, BD)
    xs_s = (x_pad * (SCALE * SCALE)).astype(np.float32)  # [5120, 192]

    kw_bf = kw_pad.astype(BF16NP)                    # [5120, 256]
    kwt_s = (kw_pad.T * SCALE).astype(FP8NP)         # [256, 5120]
    qw_bf = qw_pad.astype(BF16NP)                    # [256, 5120]

    qw8_tail = np.ascontiguousarray(qw8_full[:, 4 * OC:])
    in_maps_a = []
    for c in range(N_CORES):
        sl = slice(c * S, (c + 1) * S)
        kwt8 = np.ascontiguousarray(
            kwt_s[:, sl].reshape(2, 128, S).transpose(1, 0, 2).reshape(128, -1))
        blob0 = np.ascontiguousarray(
            np.concatenate([kwt8, qw8_full[:, 0:4 * OC]], axis=1))
        xs = np.ascontiguousarray(
            xs_s[sl].reshape(MT, 128, BD).transpose(1, 0, 2).reshape(128, -1))
        kwb = np.ascontiguousarray(
            kw_bf[sl].reshape(MT, 128, H_DIM).transpose(1, 0, 2).reshape(128, -1))
        in_maps_a.append({
            "blob0": blob0,
            "qw8": qw8_tail,
            "xs": xs,
            "kwb": kwb,
        })

    res_a, a_ns = _run_phase(nc_a, in_maps_a, profile)

    # gather: sum the 8 partial T contributions [256, 192] and apply the
    # tiny 3x3 value map to the summed intermediate (host glue alongside
    # the partial-sum reduction; all O(N^2)/O(N*H*B) math is on device)
    tsum = np.zeros((H_DIM, BD), np.float32)
    for r in res_a:
        tv = r["tv"].astype(np.float32)              # [128, 2*BD]
        tsum += tv.reshape(128, 2, BD).transpose(1, 0, 2).reshape(H_DIM, BD)
    tv3 = np.einsum("hbd,ed->hbe", tsum.reshape(H_DIM, B, 3),
                    value_w).reshape(H_DIM, BD)
    ts_in = np.ascontiguousarray(
        tv3.reshape(2, 128, BD).transpose(1, 0, 2).reshape(128, -1)
    ).astype(BF16NP)

    in_maps_b = []
    for c in range(N_CORES):
        sl = slice(c * S, (c + 1) * S)
        # ot-major: [128, ot, j, 128] -> col = ot*256 + j*128 + o_local
        qwyb = np.ascontiguousarray(
            qw_bf[:, sl].reshape(2, 128, MT, 128)
            .transpose(1, 2, 0, 3).reshape(128, -1))
        blob = np.ascontiguousarray(
            np.concatenate([ts_in, qwyb[:, 0:256]], axis=1))
        in_maps_b.append({"blobb": blob, "qwyb": np.ascontiguousarray(
            qwyb[:, 256:])})

    res_b, b_ns = _run_phase(nc_b, in_maps_b, profile)

    # unshard: y_sb [128, MT*BD] bf16 -> rows c*640 + ot*128 + p
    y_full = np.zeros((N_PAD, BD), np.float32)
    for c, r in enumerate(res_b):
        yb = np.asarray(r["y"]).astype(np.float32)
        y_full[c * S:(c + 1) * S] = (
            yb.reshape(128, MT, BD).transpose(1, 0, 2).reshape(S, BD))
    y = np.ascontiguousarray(
        y_full[:N].reshape(N, B, 3).transpose(1, 0, 2)).astype(np.float32)

    LAST_PHASE_A_NS = a_ns
    LAST_PHASE_B_NS = b_ns
    LAST_HW_EXEC_NS = (a_ns or 0) + (b_ns or 0) if profile else None
    return y
